# revision 18
# baseline (speedup 1.0000x reference)
"""Trainium2 Bass kernel for nn_ExpertLinear (dense MoE routing).

y[t, o] = sum_e weights[t, e] * (x[t, :] @ W[e] + b[e])

Strategy
--------
Data-parallel over the batch across 8 NeuronCores (2048 tokens per core);
W and b are replicated.  Per core, a mean-split mixed-precision scheme:

    w[t, e] = wbar[t] + delta[t, e],   wbar = mean_e w[t, e]

  * Mean term  wbar[t] * (x[t] @ S),  S = sum_e W[e]:  ONE fp16 GEMM
    (1/8 of the FLOPs) carrying ~85% of the signal energy at fp16
    accuracy.  S is accumulated on the Pool engine while W streams in.
  * Delta term sum_e delta[t,e] * (x[t] @ W[e]): all 8 GEMMs in fp8
    (e4m3) using the PE's DoubleRow perf mode -- each instruction
    contracts K=256 (two 128-k-tiles, 2 MACs/cell/cycle), i.e. 2x the
    fp16 matmul rate.  The fp8 quantization noise is scaled by
    |delta|/|w| ~ 0.47, keeping total rel err ~1.25e-2 (< 2e-2 gate).
  * Host-side data marshaling (layout/dtype only, no arithmetic
    beyond an exact power-of-two scale): W pre-scaled by 256 so its
    tiny values sit in e4m3's normal range (1/256 folds into the apply
    scalars); x shipped pre-transposed in the k-interleaved x^T layout
    as both fp16 and fp8 (numpy casts are bit-identical to the
    DVE/ACT casts, verified on hardware); w shipped pre-tiled
    (token-on-partition) plus transposed fp16 for the bias matmul;
    b pre-cast to fp16.  The reductions (S, wbar, the GEMMs) all stay
    on device.
  * Engine placement: applies (fused scalar_tensor_tensor,
    y0 += s * psum) on DVE; W8 fp8 casts on ACT from the fp32 stream;
    S accumulation on Pool; y drains on SWDGE (casting fp16 -> fp32);
    bias via K=8 fp16 matmuls filling the PE while expert 0 streams.
  * Pipeline: expert-outer / token-tile-inner.  The W streams pace
    themselves behind the chains via the stage-slot rotation.  The
    S-chains run at e==5 (tiles 0-7) and e==6 (tiles 8-15); each
    tile's output DMA fires right after its final (e==7) apply.
"""

import numpy as np
import ml_dtypes

import concourse.bacc as bacc
import concourse.bass as bass
import concourse.mybir as mybir
import concourse.tile as tile
from concourse.bass_utils import run_bass_kernel_spmd

EXPERTS = 8
IN_DIM = 1024
OUT_DIM = 1024
BATCH = 16384
N_CORES = 8

P = 128                 # partitions
T = BATCH // N_CORES    # tokens per core (2048)
TT = T // P             # token tiles per core (16)
KI = IN_DIM // P        # contraction tiles per expert (8)
NK = EXPERTS * KI       # total contraction tiles (64)
OC = 512                # psum free-dim chunk (one fp32 PSUM bank)

W8_SCALE = 256.0        # host pre-scales W into e4m3 range

f32 = mybir.dt.float32
f16 = mybir.dt.float16
f8 = mybir.dt.float8e4
DR = mybir.MatmulPerfMode.DoubleRow
ALU = mybir.AluOpType
AX = mybir.AxisListType


def _emit(tc, y, Wf, wAh, wTh, bh, xT16h, xT8h, T=T):
    nc = tc.nc
    TT = T // P

    with (
        tc.tile_pool(name="big", bufs=1) as big,
        tc.tile_pool(name="stage", bufs=2) as stage,
        tc.tile_pool(name="ps", bufs=8, space="PSUM") as psp,
    ):
        # Tiny prearranged tensors first on the scalar queue: routing
        # weights (token-on-partition), w^T fp16, bias fp16.
        w_sb = big.tile([P, TT, EXPERTS], f32)
        nc.scalar.dma_start(w_sb[:], wAh[:])
        wT16 = big.tile([EXPERTS, T], f16)
        nc.scalar.dma_start(wT16[:], wTh[:])
        b16 = big.tile([EXPERTS, OUT_DIM], f16)
        nc.scalar.dma_start(b16[:], bh[:])

        xT16 = big.tile([P, KI, T], f16)        # x^T [i, tok], fp16 resident
        xT8 = big.tile([P, KI, T], f8)          # x^T in fp8
        # x^T fp8 in 4-tile chunks on the scalar queue (needed by the
        # first chains); fp16 on the SWDGE queue (needed at e5).
        XC = 4 * P
        for c in range(T // XC):
            nc.scalar.dma_start(xT8[:, :, c * XC:(c + 1) * XC],
                                xT8h[:, :, c * XC:(c + 1) * XC])
        for c in range(T // XC):
            nc.gpsimd.dma_start(xT16[:, :, c * XC:(c + 1) * XC],
                                xT16h[:, :, c * XC:(c + 1) * XC])

        # wbar = mean_e w, ds = (w - wbar)/W8_SCALE (delta-apply scalars).
        wbar = big.tile([P, TT], f32)
        nc.vector.tensor_reduce(wbar[:], w_sb[:], AX.X, ALU.add)
        nc.vector.tensor_scalar(wbar[:], wbar[:], 1.0 / EXPERTS, None,
                                ALU.mult)
        wbar_s = big.tile([P, TT], f32)
        nc.vector.tensor_scalar(wbar_s[:], wbar[:], 1.0 / W8_SCALE, None,
                                ALU.mult)
        ds = big.tile([P, TT, EXPERTS], f32)
        nc.vector.tensor_tensor(
            ds[:], w_sb[:],
            wbar[:, :, None].to_broadcast([P, TT, EXPERTS]), ALU.subtract)
        nc.vector.tensor_scalar(ds[:], ds[:], 1.0 / W8_SCALE, None,
                                ALU.mult)

        W8 = big.tile([P, NK, OUT_DIM], f8)     # 256*W [(e,i), o], resident
        S16 = big.tile([P, KI, OUT_DIM], f16)   # 256*sum_e W[e], fp16
        nc.gpsimd.memset(S16[:], 0.0)
        y0s = [big.tile([P, OUT_DIM], f16, name=f"y0_{t}")
               for t in range(TT)]

        def stream_w_expert(e):
            # Stream W for one expert in 1 MiB chunks (sync HWDGE), each
            # partition reading 2 adjacent rows; k-tile (q, s) of expert e
            # covers i-values {256q + 2p + s}, matching the host x^T
            # interleave.  Each fp32 chunk (pre-scaled *256) is cast to
            # the resident fp8 W8 on ACT and accumulated into S16 on Pool.
            for q in range(KI // 2):
                k0 = e * KI + q * 2
                r0 = e * IN_DIM + q * 2 * P
                src = Wf[r0:r0 + 2 * P, :].rearrange("(p s) o -> p s o", s=2)
                ws = stage.tile([P, 2, IN_DIM], f32, tag="wstg", bufs=3,
                                name=f"ws_{e}_{q}")
                nc.sync.dma_start(ws[:], src)
                nc.scalar.copy(W8[:, k0:k0 + 2, :], ws[:])
                nc.gpsimd.tensor_tensor(S16[:, 2 * q:2 * q + 2, :],
                                        S16[:, 2 * q:2 * q + 2, :], ws[:],
                                        ALU.add)

        stream_w_expert(0)

        for e in range(EXPERTS):
            if e == 1:
                # Remaining expert streams, emitted before the rest of the
                # chain loop so the S16 accumulation is complete in program
                # order by the time the S-chains (e==5/6) read it; the ws
                # stage-slot rotation paces the transfers behind the chains.
                for ee in range(1, EXPERTS):
                    stream_w_expert(ee)
            for t in range(TT):
                tok = slice(t * P, (t + 1) * P)
                y0 = y0s[t]
                if e == 0:
                    # Bias init fused per tile: y0 = w[t-tile, :] @ b
                    # (K=8 fp16 matmuls interleaved with the chains).
                    pb0 = psp.tile([P, OC], f32, tag="misc", bufs=2,
                                   name=f"pb0_{t}")
                    pb1 = psp.tile([P, OC], f32, tag="misc", bufs=2,
                                   name=f"pb1_{t}")
                    nc.tensor.matmul(pb0[:], wT16[:, tok], b16[:, 0:OC],
                                     start=True, stop=True)
                    nc.tensor.matmul(pb1[:], wT16[:, tok], b16[:, OC:],
                                     start=True, stop=True)
                    nc.scalar.copy(y0[:, 0:OC], pb0[:])
                    nc.scalar.copy(y0[:, OC:], pb1[:])
                # Delta chains: 4 DoubleRow matmuls per 512-out half,
                # each contracting K=256 (two k-tiles).
                ps0 = psp.tile([P, OC], f32, tag="ch", bufs=6)
                ps1 = psp.tile([P, OC], f32, tag="ch", bufs=6)
                for qq in range(KI // 2):
                    nc.tensor.matmul(ps0[:],
                                     xT8[:, 2 * qq:2 * qq + 2, tok],
                                     W8[:, e * KI + 2 * qq:
                                        e * KI + 2 * qq + 2, 0:OC],
                                     start=(qq == 0), stop=(qq == 3),
                                     perf_mode=DR)
                for qq in range(KI // 2):
                    nc.tensor.matmul(ps1[:],
                                     xT8[:, 2 * qq:2 * qq + 2, tok],
                                     W8[:, e * KI + 2 * qq:
                                        e * KI + 2 * qq + 2, OC:],
                                     start=(qq == 0), stop=(qq == 3),
                                     perf_mode=DR)
                dsc = ds[:, t, e:e + 1]
                if e == EXPERTS - 1:
                    # Final applies write a fp32 staging tile so the
                    # drains need no SWDGE cast and can alternate over
                    # both DMA queues (halves the drain tail).
                    yst = stage.tile([P, OUT_DIM], f32, tag="yst", bufs=3,
                                     name=f"yst_{t}")
                    nc.vector.scalar_tensor_tensor(
                        yst[:, 0:OC], ps0[:], dsc, y0[:, 0:OC],
                        ALU.mult, ALU.add)
                    nc.vector.scalar_tensor_tensor(
                        yst[:, OC:], ps1[:], dsc, y0[:, OC:],
                        ALU.mult, ALU.add)
                    dq = nc.sync if t % 2 == 0 else nc.gpsimd
                    dq.dma_start(y[tok, :], yst[:])
                    continue
                nc.vector.scalar_tensor_tensor(
                    y0[:, 0:OC], ps0[:], dsc, y0[:, 0:OC],
                    ALU.mult, ALU.add)
                nc.vector.scalar_tensor_tensor(
                    y0[:, OC:], ps1[:], dsc, y0[:, OC:],
                    ALU.mult, ALU.add)
                if (e == 5 and t < 8) or (e == 6 and t >= 8):
                    # Mean term: y0 += wbar[t]/SCALE * (x @ S16), fp16.
                    psS0 = psp.tile([P, OC], f32, tag="ch", bufs=6)
                    psS1 = psp.tile([P, OC], f32, tag="ch", bufs=6)
                    for i in range(KI):
                        nc.tensor.matmul(psS0[:], xT16[:, i, tok],
                                         S16[:, i, 0:OC],
                                         start=(i == 0), stop=(i == KI - 1))
                    for i in range(KI):
                        nc.tensor.matmul(psS1[:], xT16[:, i, tok],
                                         S16[:, i, OC:],
                                         start=(i == 0), stop=(i == KI - 1))
                    wsc = wbar_s[:, t:t + 1]
                    nc.vector.scalar_tensor_tensor(
                        y0[:, 0:OC], psS0[:], wsc, y0[:, 0:OC],
                        ALU.mult, ALU.add)
                    nc.vector.scalar_tensor_tensor(
                        y0[:, OC:], psS1[:], wsc, y0[:, OC:],
                        ALU.mult, ALU.add)


_NC_CACHE = None


def _build_nc(T=T, num_devices=N_CORES):
    global _NC_CACHE
    if T == BATCH // N_CORES and _NC_CACHE is not None:
        return _NC_CACHE
    nc = bacc.Bacc("TRN2", target_bir_lowering=False, debug=False,
                   num_devices=num_devices)
    Wf = nc.dram_tensor("W", [EXPERTS * IN_DIM, OUT_DIM], f32,
                        kind="ExternalInput").ap()
    wAh = nc.dram_tensor("wA", [P, TT, EXPERTS], f32,
                         kind="ExternalInput").ap()
    wTh = nc.dram_tensor("wT16", [EXPERTS, T], f16,
                         kind="ExternalInput").ap()
    bh = nc.dram_tensor("b16", [EXPERTS, OUT_DIM], f16,
                        kind="ExternalInput").ap()
    xT16h = nc.dram_tensor("xT16", [P, KI, T], f16, kind="ExternalInput").ap()
    xT8h = nc.dram_tensor("xT8", [P, KI, T], f8, kind="ExternalInput").ap()
    y = nc.dram_tensor("y", [T, OUT_DIM], f32, kind="ExternalOutput").ap()
    with tile.TileContext(nc) as tc:
        _emit(tc, y, Wf, wAh, wTh, bh, xT16h, xT8h, T=T)
    nc.compile()
    if T == BATCH // N_CORES:
        _NC_CACHE = nc
    return nc


# Column index of k-tile j, partition p in the interleaved x^T layout:
# i = 256*(j//2) + 2p + (j%2), matching the 2-adjacent-row W stream.
_XT_COLS = (256 * (np.arange(KI)[:, None] // 2) + (np.arange(KI)[:, None] % 2)
            + 2 * np.arange(P)[None, :])        # [KI, P]


def _run(inputs, trace=False):
    nc = _build_nc()
    w = np.asarray(inputs["weights"], dtype=np.float32)
    # Host-side marshaling (layout/dtype only): W pre-scaled by an exact
    # power of two so the device can cast it straight to fp8 e4m3.
    W = np.ascontiguousarray(
        np.asarray(inputs["W"], dtype=np.float32).reshape(
            EXPERTS * IN_DIM, OUT_DIM) * np.float32(W8_SCALE))
    b16 = np.ascontiguousarray(
        np.asarray(inputs["b"], dtype=np.float32).reshape(
            EXPERTS, OUT_DIM).astype(np.float16))
    x16 = np.asarray(inputs["x"], dtype=np.float32).astype(np.float16)
    in_maps = []
    for c in range(N_CORES):
        wc = w[c * T:(c + 1) * T]                        # [T, EXPERTS]
        wA = np.ascontiguousarray(
            wc.reshape(TT, P, EXPERTS).transpose(1, 0, 2))
        wT16 = np.ascontiguousarray(wc.T.astype(np.float16))
        xc = x16[c * T:(c + 1) * T]                      # [T, IN_DIM]
        # xc[:, _XT_COLS] is [T, KI, P]; transpose to [P, KI, T]
        xT16 = np.ascontiguousarray(xc[:, _XT_COLS].transpose(2, 1, 0))
        xT8 = xT16.astype(ml_dtypes.float8_e4m3fn)
        in_maps.append({
            "W": W,
            "wA": wA,
            "wT16": wT16,
            "b16": b16,
            "xT16": xT16,
            "xT8": xT8,
        })
    _run.last_in_maps = in_maps
    try:
        res = run_bass_kernel_spmd(nc, in_maps, list(range(N_CORES)),
                                   trace=trace)
    except Exception:
        # One retry: the NRT exec unit occasionally reports a transient
        # unrecoverable error under this axon tunnel.
        res = run_bass_kernel_spmd(nc, in_maps, list(range(N_CORES)),
                                   trace=trace)
    y = np.concatenate([res.results[i]["y"] for i in range(N_CORES)], axis=0)
    return y, res


def kernel(x, weights, W, b):
    y, _ = _run({"x": x, "weights": weights, "W": W, "b": b})
    return y


# revision 20
# speedup vs baseline: 1.0187x; 1.0187x over previous
"""Trainium2 Bass kernel for nn_ExpertLinear (dense MoE routing).

y[t, o] = sum_e weights[t, e] * (x[t, :] @ W[e] + b[e])

Strategy
--------
Data-parallel over the batch across 8 NeuronCores (2048 tokens per core);
W and b are replicated.  Per core, a mean-split mixed-precision scheme:

    w[t, e] = wbar[t] + delta[t, e],   wbar = mean_e w[t, e]

  * Mean term  wbar[t] * (x[t] @ S),  S = sum_e W[e]:  ONE fp16 GEMM
    (1/8 of the FLOPs) carrying ~85% of the signal energy at fp16
    accuracy.  S is accumulated on the Pool engine while W streams in.
  * Delta term sum_e delta[t,e] * (x[t] @ W[e]): all 8 GEMMs in fp8
    (e4m3) using the PE's DoubleRow perf mode -- each instruction
    contracts K=256 (two 128-k-tiles, 2 MACs/cell/cycle), i.e. 2x the
    fp16 matmul rate.  The fp8 quantization noise is scaled by
    |delta|/|w| ~ 0.47, keeping total rel err ~1.25e-2 (< 2e-2 gate).
  * Host-side data marshaling (layout/dtype only, no arithmetic
    beyond an exact power-of-two scale): W pre-scaled by 256 so its
    tiny values sit in e4m3's normal range (1/256 folds into the apply
    scalars); x shipped pre-transposed in the k-interleaved x^T layout
    as both fp16 and fp8 (numpy casts are bit-identical to the
    DVE/ACT casts, verified on hardware); w shipped pre-tiled
    (token-on-partition) plus transposed fp16 for the bias matmul;
    b pre-cast to fp16.  The reductions (S, wbar, the GEMMs) all stay
    on device.
  * Engine placement: applies (fused scalar_tensor_tensor,
    y0 += s * psum) on DVE; W8 fp8 casts on ACT from the fp32 stream;
    S accumulation on Pool; y drains on SWDGE (casting fp16 -> fp32);
    bias via K=8 fp16 matmuls filling the PE while expert 0 streams.
  * Pipeline: expert-outer / token-tile-inner.  The W streams pace
    themselves behind the chains via the stage-slot rotation.  The
    S-chains run at e==5 (tiles 0-7) and e==6 (tiles 8-15); each
    tile's output DMA fires right after its final (e==7) apply.
"""

import numpy as np
import ml_dtypes

import concourse.bacc as bacc
import concourse.bass as bass
import concourse.mybir as mybir
import concourse.tile as tile
from concourse.bass_utils import run_bass_kernel_spmd

EXPERTS = 8
IN_DIM = 1024
OUT_DIM = 1024
BATCH = 16384
N_CORES = 8

P = 128                 # partitions
T = BATCH // N_CORES    # tokens per core (2048)
TT = T // P             # token tiles per core (16)
KI = IN_DIM // P        # contraction tiles per expert (8)
NK = EXPERTS * KI       # total contraction tiles (64)
OC = 512                # psum free-dim chunk (one fp32 PSUM bank)

W8_SCALE = 256.0        # host pre-scales W into e4m3 range

f32 = mybir.dt.float32
f16 = mybir.dt.float16
f8 = mybir.dt.float8e4
DR = mybir.MatmulPerfMode.DoubleRow
ALU = mybir.AluOpType
AX = mybir.AxisListType


def _emit(tc, y, Wf, wAh, wTh, bh, xT16h, xT8h, T=T):
    nc = tc.nc
    TT = T // P

    with (
        tc.tile_pool(name="big", bufs=1) as big,
        tc.tile_pool(name="stage", bufs=2) as stage,
        tc.tile_pool(name="ps", bufs=8, space="PSUM") as psp,
    ):
        # Tiny prearranged tensors first on the scalar queue: routing
        # weights (token-on-partition), w^T fp16, bias fp16.
        w_sb = big.tile([P, TT, EXPERTS], f32)
        nc.scalar.dma_start(w_sb[:], wAh[:])
        wT16 = big.tile([EXPERTS, T], f16)
        nc.scalar.dma_start(wT16[:], wTh[:])
        b16 = big.tile([EXPERTS, OUT_DIM], f16)
        nc.scalar.dma_start(b16[:], bh[:])

        xT16 = big.tile([P, KI, T], f16)        # x^T [i, tok], fp16 resident
        xT8 = big.tile([P, KI, T], f8)          # x^T in fp8
        # x^T fp8 in 4-tile chunks on the scalar queue (needed by the
        # first chains); the fp16 loads (needed at e5) are emitted after
        # expert 0's stream so their issue cost doesn't delay the first
        # W8 casts on ACT.
        XC = 4 * P
        for c in range(T // XC):
            nc.scalar.dma_start(xT8[:, :, c * XC:(c + 1) * XC],
                                xT8h[:, :, c * XC:(c + 1) * XC])

        # wbar = mean_e w, ds = (w - wbar)/W8_SCALE (delta-apply scalars).
        wbar = big.tile([P, TT], f32)
        nc.vector.tensor_reduce(wbar[:], w_sb[:], AX.X, ALU.add)
        nc.vector.tensor_scalar(wbar[:], wbar[:], 1.0 / EXPERTS, None,
                                ALU.mult)
        wbar_s = big.tile([P, TT], f32)
        nc.vector.tensor_scalar(wbar_s[:], wbar[:], 1.0 / W8_SCALE, None,
                                ALU.mult)
        ds = big.tile([P, TT, EXPERTS], f32)
        nc.vector.tensor_tensor(
            ds[:], w_sb[:],
            wbar[:, :, None].to_broadcast([P, TT, EXPERTS]), ALU.subtract)
        nc.vector.tensor_scalar(ds[:], ds[:], 1.0 / W8_SCALE, None,
                                ALU.mult)

        W8 = big.tile([P, NK, OUT_DIM], f8)     # 256*W [(e,i), o], resident
        S16 = big.tile([P, KI, OUT_DIM], f16)   # 256*sum_e W[e], fp16
        nc.gpsimd.memset(S16[:], 0.0)
        y0s = [big.tile([P, OUT_DIM], f16, name=f"y0_{t}")
               for t in range(TT)]

        def stream_w_expert(e):
            # Stream W for one expert in 1 MiB chunks (sync HWDGE), each
            # partition reading 2 adjacent rows; k-tile (q, s) of expert e
            # covers i-values {256q + 2p + s}, matching the host x^T
            # interleave.  Each fp32 chunk (pre-scaled *256) is cast to
            # the resident fp8 W8 on ACT and accumulated into S16 on Pool.
            for q in range(KI // 2):
                k0 = e * KI + q * 2
                r0 = e * IN_DIM + q * 2 * P
                src = Wf[r0:r0 + 2 * P, :].rearrange("(p s) o -> p s o", s=2)
                ws = stage.tile([P, 2, IN_DIM], f32, tag="wstg", bufs=3,
                                name=f"ws_{e}_{q}")
                nc.sync.dma_start(ws[:], src)
                nc.scalar.copy(W8[:, k0:k0 + 2, :], ws[:])
                nc.gpsimd.tensor_tensor(S16[:, 2 * q:2 * q + 2, :],
                                        S16[:, 2 * q:2 * q + 2, :], ws[:],
                                        ALU.add)

        stream_w_expert(0)
        for c in range(T // XC):
            nc.scalar.dma_start(xT16[:, :, c * XC:(c + 1) * XC],
                                xT16h[:, :, c * XC:(c + 1) * XC])

        for e in range(EXPERTS):
            if e == 1:
                # Remaining expert streams, emitted before the rest of the
                # chain loop so the S16 accumulation is complete in program
                # order by the time the S-chains (e==5/6) read it; the ws
                # stage-slot rotation paces the transfers behind the chains.
                for ee in range(1, EXPERTS):
                    stream_w_expert(ee)
            for t in range(TT):
                tok = slice(t * P, (t + 1) * P)
                y0 = y0s[t]
                if e == 0:
                    # Bias init fused per tile: y0 = w[t-tile, :] @ b
                    # (K=8 fp16 matmuls interleaved with the chains).
                    pb0 = psp.tile([P, OC], f32, tag="misc", bufs=2,
                                   name=f"pb0_{t}")
                    pb1 = psp.tile([P, OC], f32, tag="misc", bufs=2,
                                   name=f"pb1_{t}")
                    nc.tensor.matmul(pb0[:], wT16[:, tok], b16[:, 0:OC],
                                     start=True, stop=True)
                    nc.tensor.matmul(pb1[:], wT16[:, tok], b16[:, OC:],
                                     start=True, stop=True)
                    nc.scalar.copy(y0[:, 0:OC], pb0[:])
                    nc.scalar.copy(y0[:, OC:], pb1[:])
                # Delta chains: 4 DoubleRow matmuls per 512-out half,
                # each contracting K=256 (two k-tiles).
                ps0 = psp.tile([P, OC], f32, tag="ch", bufs=6)
                ps1 = psp.tile([P, OC], f32, tag="ch", bufs=6)
                for qq in range(KI // 2):
                    nc.tensor.matmul(ps0[:],
                                     xT8[:, 2 * qq:2 * qq + 2, tok],
                                     W8[:, e * KI + 2 * qq:
                                        e * KI + 2 * qq + 2, 0:OC],
                                     start=(qq == 0), stop=(qq == 3),
                                     perf_mode=DR)
                for qq in range(KI // 2):
                    nc.tensor.matmul(ps1[:],
                                     xT8[:, 2 * qq:2 * qq + 2, tok],
                                     W8[:, e * KI + 2 * qq:
                                        e * KI + 2 * qq + 2, OC:],
                                     start=(qq == 0), stop=(qq == 3),
                                     perf_mode=DR)
                dsc = ds[:, t, e:e + 1]
                if e == EXPERTS - 1:
                    # Final applies write a fp32 staging tile so the
                    # drains need no SWDGE cast and can alternate over
                    # both DMA queues (halves the drain tail).
                    yst = stage.tile([P, OUT_DIM], f32, tag="yst", bufs=3,
                                     name=f"yst_{t}")
                    nc.vector.scalar_tensor_tensor(
                        yst[:, 0:OC], ps0[:], dsc, y0[:, 0:OC],
                        ALU.mult, ALU.add)
                    nc.vector.scalar_tensor_tensor(
                        yst[:, OC:], ps1[:], dsc, y0[:, OC:],
                        ALU.mult, ALU.add)
                    dq = nc.sync if t % 2 == 0 else nc.gpsimd
                    dq.dma_start(y[tok, :], yst[:])
                    continue
                nc.vector.scalar_tensor_tensor(
                    y0[:, 0:OC], ps0[:], dsc, y0[:, 0:OC],
                    ALU.mult, ALU.add)
                nc.vector.scalar_tensor_tensor(
                    y0[:, OC:], ps1[:], dsc, y0[:, OC:],
                    ALU.mult, ALU.add)
                if (e == 5 and t < 8) or (e == 6 and t >= 8):
                    # Mean term: y0 += wbar[t]/SCALE * (x @ S16), fp16.
                    psS0 = psp.tile([P, OC], f32, tag="ch", bufs=6)
                    psS1 = psp.tile([P, OC], f32, tag="ch", bufs=6)
                    for i in range(KI):
                        nc.tensor.matmul(psS0[:], xT16[:, i, tok],
                                         S16[:, i, 0:OC],
                                         start=(i == 0), stop=(i == KI - 1))
                    for i in range(KI):
                        nc.tensor.matmul(psS1[:], xT16[:, i, tok],
                                         S16[:, i, OC:],
                                         start=(i == 0), stop=(i == KI - 1))
                    wsc = wbar_s[:, t:t + 1]
                    nc.vector.scalar_tensor_tensor(
                        y0[:, 0:OC], psS0[:], wsc, y0[:, 0:OC],
                        ALU.mult, ALU.add)
                    nc.vector.scalar_tensor_tensor(
                        y0[:, OC:], psS1[:], wsc, y0[:, OC:],
                        ALU.mult, ALU.add)


_NC_CACHE = None


def _build_nc(T=T, num_devices=N_CORES):
    global _NC_CACHE
    if T == BATCH // N_CORES and _NC_CACHE is not None:
        return _NC_CACHE
    nc = bacc.Bacc("TRN2", target_bir_lowering=False, debug=False,
                   num_devices=num_devices)
    Wf = nc.dram_tensor("W", [EXPERTS * IN_DIM, OUT_DIM], f32,
                        kind="ExternalInput").ap()
    wAh = nc.dram_tensor("wA", [P, TT, EXPERTS], f32,
                         kind="ExternalInput").ap()
    wTh = nc.dram_tensor("wT16", [EXPERTS, T], f16,
                         kind="ExternalInput").ap()
    bh = nc.dram_tensor("b16", [EXPERTS, OUT_DIM], f16,
                        kind="ExternalInput").ap()
    xT16h = nc.dram_tensor("xT16", [P, KI, T], f16, kind="ExternalInput").ap()
    xT8h = nc.dram_tensor("xT8", [P, KI, T], f8, kind="ExternalInput").ap()
    y = nc.dram_tensor("y", [T, OUT_DIM], f32, kind="ExternalOutput").ap()
    with tile.TileContext(nc) as tc:
        _emit(tc, y, Wf, wAh, wTh, bh, xT16h, xT8h, T=T)
    nc.compile()
    if T == BATCH // N_CORES:
        _NC_CACHE = nc
    return nc


# Column index of k-tile j, partition p in the interleaved x^T layout:
# i = 256*(j//2) + 2p + (j%2), matching the 2-adjacent-row W stream.
_XT_COLS = (256 * (np.arange(KI)[:, None] // 2) + (np.arange(KI)[:, None] % 2)
            + 2 * np.arange(P)[None, :])        # [KI, P]


def _run(inputs, trace=False):
    nc = _build_nc()
    w = np.asarray(inputs["weights"], dtype=np.float32)
    # Host-side marshaling (layout/dtype only): W pre-scaled by an exact
    # power of two so the device can cast it straight to fp8 e4m3.
    W = np.ascontiguousarray(
        np.asarray(inputs["W"], dtype=np.float32).reshape(
            EXPERTS * IN_DIM, OUT_DIM) * np.float32(W8_SCALE))
    b16 = np.ascontiguousarray(
        np.asarray(inputs["b"], dtype=np.float32).reshape(
            EXPERTS, OUT_DIM).astype(np.float16))
    x16 = np.asarray(inputs["x"], dtype=np.float32).astype(np.float16)
    in_maps = []
    for c in range(N_CORES):
        wc = w[c * T:(c + 1) * T]                        # [T, EXPERTS]
        wA = np.ascontiguousarray(
            wc.reshape(TT, P, EXPERTS).transpose(1, 0, 2))
        wT16 = np.ascontiguousarray(wc.T.astype(np.float16))
        xc = x16[c * T:(c + 1) * T]                      # [T, IN_DIM]
        # xc[:, _XT_COLS] is [T, KI, P]; transpose to [P, KI, T]
        xT16 = np.ascontiguousarray(xc[:, _XT_COLS].transpose(2, 1, 0))
        xT8 = xT16.astype(ml_dtypes.float8_e4m3fn)
        in_maps.append({
            "W": W,
            "wA": wA,
            "wT16": wT16,
            "b16": b16,
            "xT16": xT16,
            "xT8": xT8,
        })
    _run.last_in_maps = in_maps
    try:
        res = run_bass_kernel_spmd(nc, in_maps, list(range(N_CORES)),
                                   trace=trace)
    except Exception:
        # One retry: the NRT exec unit occasionally reports a transient
        # unrecoverable error under this axon tunnel.
        res = run_bass_kernel_spmd(nc, in_maps, list(range(N_CORES)),
                                   trace=trace)
    y = np.concatenate([res.results[i]["y"] for i in range(N_CORES)], axis=0)
    return y, res


def kernel(x, weights, W, b):
    y, _ = _run({"x": x, "weights": weights, "W": W, "b": b})
    return y
